# revision 2
# baseline (speedup 1.0000x reference)
"""TRN2 Bass kernel for nn_BeyazKusAIAttention_36515811951168.

Key reduction: the reference applies softmax over a size-1 axis, which is
identically 1.0, so attention weights are exactly 1 and the module collapses
to
    y = (x @ Wv^T) @ Wfold^T,  with  Wfold = Wo.reshape(4096,4,1024).sum(1)
(q/rope/scores/mask are dead code; `out` is v tiled over the 4 heads, and the
o-projection of the tiled v folds head-wise into Wfold).  This is a 5x FLOP
reduction vs the reference graph.

Execution: data-parallel over the 16384 = batch*seq rows across 8 NeuronCores
(no collectives).  MM1 runs in fp16 (halves the dominant x/Wv DMA stream;
fp16 multiply with fp32 PSUM accumulation); MM2 runs in float32r (TF32-class).
Measured end-to-end relative error vs the fp32 reference is ~3.3e-4.

Per-core program (R = 2048 rows), fully fused per 512-row chunk:
  MM1: v^T = Wv @ x^T  - Wv^T k-slices streamed from HBM, x^T slices
       streamed, K=4096 accumulated across all 8 PSUM banks (one per
       128-wide v^T tile), then evicted by DVE into SBUF as float32r.
  MM2: y = v @ Wfold^T - Wfold^T resident in SBUF (16 MB), v^T tiles are
       the stationary operand straight from SBUF (no DRAM round-trip),
       K=1024 in 8 k-tiles, free-dim 512; PSUM banks shared with MM1 by
       tag rotation.

Host-side layouts (partition dim = contraction dim for both matmuls):
  xt [32,128,R]: xt[k,p,r] = x[row r, dim 128k+p]     (transposed shard)
  wvt[32,128,1024]: wvt[k,p,m] = Wv[m, 128k+p]
  wft[8,128,4096]:  wft[k,p,n] = Wfold[n, 128k+p]
  y  [R/128,128,4096]: y[t,p,n] = out[row 128t+p, n]
"""
import numpy as np
import concourse.bass as bass
from concourse import bacc
import concourse.mybir as mybir
from concourse.tile import TileContext
from concourse.bass_utils import run_bass_kernel_spmd

DIM = 4096
KV = 1024
N_CORES = 8
ROWS_TOTAL = 4 * 4096
ROWS = ROWS_TOTAL // N_CORES   # 2048
KT1 = DIM // 128               # 32 k-tiles, phase 1
MT1 = KV // 128                # 8 vcol tiles
KT2 = KV // 128                # 8 k-tiles, phase 2
NC2 = DIM // 512               # 8 ycol chunks
CHUNK1 = 256                   # phase-1 row-chunk width

_nc_cache = {}


def _build(rows=ROWS, loop_n=1):
    """Fused single-pass program: per 512-row chunk, MM1 (k-streamed Wv
    slices into all 8 PSUM banks) -> DVE eviction to SBUF v^T tiles -> MM2
    against resident Wfold^T; PSUM banks shared between the two matmul
    groups by tag rotation.  v never leaves SBUF; Wv^T is re-streamed per
    chunk (4x16MB), which fits under the PE-time DMA budget.

    loop_n > 1 wraps the whole program in a hardware loop — used only by
    the loop-slope timing harness (test.py), never by kernel().
    """
    CH = 512
    nch = rows // CH
    f32, f32r = mybir.dt.float32, mybir.dt.float32r
    f16 = mybir.dt.float16

    nc = bacc.Bacc(None, target_bir_lowering=False)
    XT = nc.dram_tensor("xt", [KT1, 128, rows], f16, kind="ExternalInput")
    WVT = nc.dram_tensor("wvt", [KT1, 128, KV], f16, kind="ExternalInput")
    WFT = nc.dram_tensor("wft", [KT2, 128, DIM], f32r, kind="ExternalInput")
    Y = nc.dram_tensor("y", [rows // 128, 128, DIM], f32,
                       kind="ExternalOutput")

    with TileContext(nc) as tc:
        with (
            tc.tile_pool(name="wf", bufs=1) as wfpool,
            tc.tile_pool(name="wvs", bufs=12) as wvpool,
            tc.tile_pool(name="xts", bufs=16) as xtpool,
            tc.tile_pool(name="vss", bufs=2) as vspool,
            tc.tile_pool(name="yst", bufs=2) as ypool,
            tc.tile_pool(name="ps", bufs=1, space="PSUM") as pspool,
        ):
            def body():
                wf = []
                for n in range(NC2):
                    wfn = wfpool.tile([128, KT2, 512], f32r, tag=f"wf{n}")
                    for k in range(KT2):
                        nc.sync.dma_start(wfn[:, k, :],
                                          WFT[k, :, n * 512:(n + 1) * 512])
                    wf.append(wfn)
                for rc in range(nch):
                    ps1 = [pspool.tile([128, CH], f32, tag=f"ps{m}",
                                       name=f"ps1_{rc}_{m}")
                           for m in range(MT1)]
                    for k in range(KT1):
                        wvk = wvpool.tile([128, KV], f16, tag="wv")
                        nc.sync.dma_start(wvk[:], WVT[k])
                        xtk = xtpool.tile([128, CH], f16, tag="xt")
                        nc.sync.dma_start(xtk[:],
                                          XT[k, :, rc * CH:(rc + 1) * CH])
                        for m in range(MT1):
                            nc.tensor.matmul(
                                ps1[m][:], wvk[:, m * 128:(m + 1) * 128],
                                xtk[:], start=(k == 0), stop=(k == KT1 - 1))
                    vs = []
                    for m in range(MT1):
                        v = vspool.tile([128, CH], f32r, tag=f"vs{m}")
                        nc.vector.tensor_copy(v[:], ps1[m][:])
                        vs.append(v)
                    for sub in range(CH // 128):
                        for n in range(NC2):
                            ps2 = pspool.tile(
                                [128, 512], f32,
                                tag=f"ps{(sub * NC2 + n) % MT1}")
                            for k2 in range(KT2):
                                nc.tensor.matmul(
                                    ps2[:],
                                    vs[k2][:, sub * 128:(sub + 1) * 128],
                                    wf[n][:, k2, :],
                                    start=(k2 == 0), stop=(k2 == KT2 - 1))
                            ys = ypool.tile([128, 512], f32, tag="ys")
                            nc.vector.tensor_copy(ys[:], ps2[:])
                            nc.sync.dma_start(
                                Y[rc * (CH // 128) + sub, :,
                                  n * 512:(n + 1) * 512], ys[:])
            if loop_n == 1:
                body()
            else:
                with tc.For_i(0, loop_n):
                    body()
    nc.compile()
    return nc


def kernel(x, Wq, Wk, Wv, Wo, mask):
    x = np.asarray(x)
    Wv = np.asarray(Wv, dtype=np.float32)
    Wo = np.asarray(Wo, dtype=np.float32)
    B, S, D = x.shape
    assert D == DIM and B * S == ROWS_TOTAL

    # host-side relayout: transpose x once, fold Wo over heads
    x2 = np.ascontiguousarray(
        x.reshape(ROWS_TOTAL, DIM).T).astype(np.float16)
    xt_all = x2.reshape(KT1, 128, ROWS_TOTAL)
    wvt = np.ascontiguousarray(Wv.T).astype(np.float16).reshape(KT1, 128, KV)
    wfold = Wo.reshape(DIM, 4, KV).sum(axis=1)
    wft = np.ascontiguousarray(wfold.T).reshape(KT2, 128, DIM)

    in_maps = []
    for c in range(N_CORES):
        in_maps.append({
            "xt": np.ascontiguousarray(
                xt_all[:, :, c * ROWS:(c + 1) * ROWS]),
            "wvt": wvt,
            "wft": wft,
        })

    if "nc" not in _nc_cache:
        _nc_cache["nc"] = _build()
    nc = _nc_cache["nc"]

    # transient NRT device errors (e.g. NRT_EXEC_UNIT_UNRECOVERABLE right
    # after another process released the cores) succeed on retry
    last_err = None
    for _attempt in range(3):
        try:
            results = run_bass_kernel_spmd(
                nc, in_maps, core_ids=list(range(N_CORES))).results
            break
        except Exception as e:  # noqa: BLE001
            last_err = e
    else:
        raise last_err
    shards = [r["y"].reshape(ROWS, DIM) for r in results]
    out = np.concatenate(shards, axis=0).reshape(B, S, DIM)
    return out.astype(np.float32, copy=False)

